# revision 9
# baseline (speedup 1.0000x reference)
"""Trainium2 Bass kernel for nn_AxialBlock (3-axis axial attention sum).

Problem (hardcoded): x (B=4, C=512, T=16, H=32, W=32) fp32, three axial
MHA blocks (attend along W, H, T; n_head=8, d=64) each with their own
QKVO projections; outputs summed. Output (B, C, T, H, W) fp32.

Sharding: 8 cores = (batch b in 0..3) x (H-half j in 0..1). Every pass is
computed fully locally (no collectives):
  - w-pass / t-pass: tokens (t, h in owned half, w), fully local.
  - h-pass: attention along H needs all H, so the full batch sample is
    recomputed on both cores of a pair; each core keeps only its owned
    H-half of the output. (For odd cores the H axis is rotated host-side
    so the owned half is always h-positions 0..15 — attention along H is
    permutation-equivariant, so this is exact.)

On-device layout trick: x is channels-first, i.e. already "x^T" (C on
partitions) which is what the PE wants for the QKV projections. The host
pre-permutes x into three token orders (w-fastest / t-fastest / h-fastest)
so that each axial attention acts on 32 consecutive tokens ("rows").

HW cost model (measured via microbench on these devices): a self-loading
(K=128, M=128, N=512) bf16 matmul costs ~318 ns back-to-back = stream
(512/2.4GHz) + an exposed weight-load; repeating the SAME stationary drops
it to ~260 ns. So the kernel processes token tiles in PAIRS ("supertile"),
emitting the two tiles' matmuls back-to-back per weight chunk so every
other projection matmul gets the cheap repeated-stationary rate. The
out-projection is likewise paired across the two deferred tails.

Per 512-token tile (16 rows x 32 tokens):
  q^T (feat-partition) and v (token-partition) projections in bf16; k is
  evacuated parity-split into persistent pre-zeroed "kz" buffers (one head
  per 64 d-rows, rest zero) so attention scores can contract over all 128
  partitions - the PE array tiling positions with BOTH row!=0 and col!=0
  hard-crash the device (NRT_EXEC_UNIT_UNRECOVERABLE), so only (0, col) /
  (row, 0) tiles are usable. Scores: one (K=128, M=32, N=64) matmul per
  (chunk, row) computing both heads of the chunk at col-tile (0, 32j).
  Softmax is batched per 2 row groups with one op per step: exp on
  ScalarE, per-block reduce + reciprocal on VectorE, and the broadcast
  normalize on GpSimd (measured 4x faster there than on VectorE). The
  t-pass cross-fiber mask is a rank-2 matmul (-60 additive) accumulated
  under the scores before exp; the h-pass zero-fill of unowned query rows
  is a rank-1 zero matmul (PSUM accumulation base trick).
  A -> A^T via the full-width DVE 32x32 block transpose, then per-row
  contiguous (32, 512) DVE copies form a block-diagonal A^T ("abd") in
  persistent zeroed buffers (8-deep rotation); o^T = V^T @ abd lands
  feat-partition directly as one (K=128, M=64, N=4*qm) matmul per
  (chunk, head); then the out-projection.

PSUM fits exactly 8 banks: ps(2) + sps(2) + otp(2) + yps(2). To keep the
paired projections from stalling on bank WAR, every full-bank PSUM
evacuation is split into two free-dim halves running concurrently on
ScalarE and VectorE (halves the bank-release latency).

Software pipelining: the O + out-projection stage ("tails") of a
supertile is emitted AFTER the scores/softmax of the NEXT supertile, so
the PE fills the softmax-chain latency with the next supertile's
projection matmuls. The last supertile's tails of each pass spill into
the next pass (flushed at the end).

y accumulation lives entirely in SBUF as bf16 (4 chunk tiles of
(128, 8192)): the w-pass writes (with folded bias), t/h passes do strided
in-place DVE adds; one DMA writeout per chunk at the very end. This
removes the 5x16.8MB DRAM read-modify-write traffic of the 3-pass sum.

t-axis has seq len 16: two t-fibers are packed into one 32-token row with
the rank-2 mask zeroing cross-fiber attention. h-pass computes only the
owned half of the queries (qm=16 per row, packed), halving its q
projection, scores, o^T and out-projection work.
"""

import contextlib

import ml_dtypes
import numpy as np

import concourse.bass as bass
import concourse.tile as tile
from concourse import bacc, mybir
from concourse.bass_utils import run_bass_kernel_spmd

BF16 = mybir.dt.bfloat16
FP32 = mybir.dt.float32
BF16_NP = np.dtype(ml_dtypes.bfloat16)

B, C, T, H, W = 4, 512, 16, 32, 32
NH, D = 8, 64
HL = H // 2              # per-core H slice
N_CORES = 8
TOK_LOCAL = T * HL * W   # 8192 tokens owned per core
TOK_FULL = T * H * W     # 16384 tokens in a batch sample
TILE = 512               # tokens per on-chip tile
NCH = C // 128           # 4 partition chunks of the feature dim

# dev knob: cap supertiles per pass (None = full problem). Truncated builds
# are only for fast AP/scheduling smoke tests - output is wrong when set.
NSUPER_CAP = None
# dev knob: ablations for HW time attribution (output wrong when set):
#   "attn"    - skip S matmuls, softmax and O matmuls (out-proj reads v)
#   "softmax" - keep S and O matmuls, skip the softmax/transpose chain
ABLATE = None


def _split_evac(nc, dst, src, n):
    """Evacuate a PSUM tile into SBUF with the free dim split across
    ScalarE and VectorE so the bank frees in half the time."""
    h = n // 2
    nc.scalar.copy(dst[:, 0:h], src[:, 0:h])
    nc.vector.tensor_copy(dst[:, h:n], src[:, h:n])


def _build_pass(tc, pools, axis, x_ap, w_aps, y_sb, bias_aps, tml_sb, tmr_sb,
                kz_tiles, abd_tiles, pending):
    """Emit one axial-attention pass (supertile pairs, deferred tails).

    axis: 'w' | 't' | 'h'.  x_ap: (512, ntok) bf16 DRAM, token order chosen
    so each 32-token group is one attention row.  y_sb: list of NCH
    persistent (128, 8192) bf16 SBUF accumulator tiles (natural
    (t, h_local, w) token order).  pending: list holding the deferred
    tail-pair closure of the previous supertile (possibly from the
    previous pass).
    """
    nc = tc.nc
    wq_sb, wk_sb, wv_sb, wo_sb = w_aps
    ntok = TOK_FULL if axis == "h" else TOK_LOCAL
    nsuper = ntok // TILE // 2
    if NSUPER_CAP is not None:
        nsuper = min(nsuper, NSUPER_CAP)

    (xt_pool, qk_pool, v_pool, a_pool, sm_pool,
     ot_pool, ps_pool, sps_pool) = pools

    qw = TILE // 2 if axis == "h" else TILE
    otw = TILE // 2 if axis == "h" else TILE
    qm = 16 if axis == "h" else 32   # query rows kept per 32-token row
    GW = NH * 32                     # 256 free columns per row group

    def q_rhs(xt, kc):
        if qw == TILE:
            return xt[:, kc, :]
        return xt[:, kc, :].rearrange("p (a b) -> p a b", a=16)[:, :, 0:HL]

    for su in range(nsuper):
        pair = (2 * su, 2 * su + 1)
        # ---- load the two x^T tiles: (128, NCH, TILE) bf16 each
        xts = []
        for it in pair:
            xt = xt_pool.tile([128, NCH, TILE], BF16, tag="xt")
            for kc in range(NCH):
                nc.sync.dma_start(
                    xt[:, kc, :],
                    x_ap[128 * kc:128 * (kc + 1), it * TILE:(it + 1) * TILE],
                )
            xts.append(xt)

        # ---- q^T / k^T projections, PAIRED on the weight stationary:
        # for each (w chunk) the two tiles' matmuls run back-to-back so the
        # second reuses the loaded stationary (~260 vs ~318 ns measured).
        # h-pass: q only for the owned h-half (packed, N=256); k full.
        # k is evacuated parity-split into the persistent pre-zeroed kz
        # buffers (head p's 64 d-rows in place, other 64 rows zero).
        q_sbs = [qk_pool.tile([128, NCH, qw], BF16, tag="q", name="q")
                 for _ in pair]
        kzs = [kz_tiles[0], kz_tiles[1]]
        for w_sb, nw, ev in ((wq_sb, qw, 0), (wk_sb, TILE, 1)):
            for mc in range(NCH):
                # alternate between the ps banks and the (idle during the
                # projection phase) sps banks to double the WAR distance
                pool = ps_pool if mc % 2 == 0 else sps_pool
                ptag = "ps" if mc % 2 == 0 else "sps"
                pss = [pool.tile([128, TILE], FP32, tag=ptag, name=ptag,
                                 bufs=2) for _ in pair]
                for kc in range(NCH):
                    for xt, ps in zip(xts, pss):
                        nc.tensor.matmul(
                            ps[0:128, 0:nw],
                            lhsT=w_sb[:, kc, 128 * mc:128 * (mc + 1)],
                            rhs=(q_rhs(xt, kc) if nw == qw and axis == "h"
                                 else xt[:, kc, :]),
                            start=(kc == 0), stop=(kc == NCH - 1),
                        )
                for ti, ps in enumerate(pss):
                    if ev == 0:
                        _split_evac(nc, q_sbs[ti][:, mc, :], ps, nw)
                    else:
                        # parity split: latency-parallel on ACT + DVE
                        nc.scalar.copy(kzs[ti][0:64, 0, mc, :], ps[0:64, :])
                        nc.vector.tensor_copy(kzs[ti][64:128, 1, mc, :],
                                              ps[64:128, :])

        # ---- per tile: v projection (token-partition; x slices are the
        # stationary, no cross-tile reuse possible), then scores + softmax.
        # Interleaving v_b between softmax_a and S_b spaces the DVE softmax
        # chains apart so the in-order DVE queue doesn't pile up.
        tails = []
        for ti, it in enumerate(pair):
            q_sb, kz_sb, xt = q_sbs[ti], kzs[ti], xts[ti]
            v_sb = v_pool.tile([128, NCH, C], BF16, tag="v", name="v")
            for ts in range(NCH):
                ps = ps_pool.tile([128, TILE], FP32, tag="ps", bufs=2)
                for kc in range(NCH):
                    nc.tensor.matmul(
                        ps[:],
                        lhsT=xt[:, kc, 128 * ts:128 * (ts + 1)],
                        rhs=wv_sb[:, kc, :],
                        start=(kc == 0), stop=(kc == NCH - 1),
                    )
                _split_evac(nc, v_sb[:, ts, :], ps, TILE)
            abd_by_g = {}
            if ABLATE != "attn":
                # scores + softmax at 2-rowgroup granularity: S psum
                # (128, 512) = one bank; free = (g%2)*256 + headslot*32+kpos
                for gg in range(2):
                    sps = sps_pool.tile([128, 2 * GW], FP32, tag="sps",
                                        bufs=2)
                    # Accumulation-base matmul FIRST (start=True, full
                    # width): t = rank-2 cross-fiber -60 mask; h = rank-1
                    # zeros for the unwritten query rows; w = none.
                    base = axis != "w"
                    if axis == "t":
                        nc.tensor.matmul(
                            sps[:], lhsT=tml_sb[:], rhs=tmr_sb[:],
                            start=True, stop=False, skip_group_check=True,
                        )
                    elif axis == "h":
                        nc.tensor.matmul(
                            sps[:], lhsT=tc._z_sb[:, 0:128], rhs=tc._z_sb[:],
                            start=True, stop=False, skip_group_check=True,
                        )
                    nmm = 32
                    i_mm = 0
                    for gh in range(2):
                        g = 2 * gg + gh
                        for c in range(NCH):
                            for j in range(4):
                                qcol = (g * 4 + j) * qm
                                i_mm += 1
                                nc.tensor.matmul(
                                    sps[32 * j:32 * j + qm,
                                        gh * GW + 2 * c * 32:
                                        gh * GW + (2 * c + 2) * 32],
                                    lhsT=q_sb[:, c, qcol:qcol + qm],
                                    rhs=kz_sb[:, :, c,
                                              (g * 4 + j) * 32:
                                              (g * 4 + j) * 32 + 32],
                                    tile_position=(0, 32 * j),
                                    start=(not base),
                                    stop=(base and i_mm == nmm),
                                    skip_group_check=True,
                                )
                    if ABLATE == "softmax":
                        abd_by_g[2 * gg] = abd_tiles[gg % 2]
                        abd_by_g[2 * gg + 1] = abd_tiles[gg % 2]
                        continue
                    # softmax along k, one op per step per 2 row groups
                    a_sb = a_pool.tile([128, 2 * GW], BF16, tag="a")
                    nc.scalar.activation(a_sb[:], sps[:],
                                         mybir.ActivationFunctionType.Exp)
                    a3 = a_sb[:].rearrange("p (n k) -> p n k", n=2 * NH)
                    sums = sm_pool.tile([128, 2 * NH], FP32, tag="sums")
                    nc.vector.tensor_reduce(
                        sums[:], a3, axis=mybir.AxisListType.X,
                        op=mybir.AluOpType.add
                    )
                    recip = sm_pool.tile([128, 2 * NH], FP32, tag="recip")
                    nc.vector.reciprocal(recip[:], sums[:])
                    # normalize on GpSimd (measured ~4x faster than DVE for
                    # the broadcast multiply), freeing VectorE
                    nc.gpsimd.tensor_tensor(
                        a3, a3,
                        recip[:].unsqueeze(2).broadcast_to((128, 2 * NH, 32)),
                        mybir.AluOpType.mult,
                    )
                    # A -> A^T (DVE 32x32 block transpose, full width), then
                    # per-row contiguous DVE copies into the block-diagonal
                    # a_bd buffer (off-diagonal partitions stay zero from
                    # the one-time memset)
                    at_sb = a_pool.tile([128, 2 * GW], BF16, tag="at")
                    nc.vector.transpose(at_sb[:], a_sb[:])
                    abd = abd_tiles[tc._abd_flip]
                    tc._abd_flip = (tc._abd_flip + 1) % 8
                    for j in range(4):
                        eng = nc.vector.tensor_copy if j % 2 == 0 else nc.scalar.copy
                        eng(
                            abd[32 * j:32 * (j + 1), 512 * j:512 * (j + 1)],
                            at_sb[32 * j:32 * (j + 1), :],
                        )
                    abd_by_g[2 * gg] = abd
                    abd_by_g[2 * gg + 1] = abd
            tails.append((it, v_sb, abd_by_g))

        # ---- flush the previous supertile's tails now that this
        # supertile's scores are queued
        if pending:
            pending.pop(0)()

        def tail_pair(tails=tails, otw=otw, qm=qm, wo_sb=wo_sb, axis=axis):
            # o^T = V^T A_bd per tile, chunk-outer; evacuate each chunk as
            # it completes (free-dim split across ACT+DVE)
            ot_sbs = []
            for it, v_sb, abd_by_g in tails:
                ot_sb = ot_pool.tile([128, NCH, otw], BF16, tag="ot")
                if ABLATE == "attn":
                    for c in range(NCH):
                        nc.gpsimd.tensor_copy(ot_sb[:, c, :],
                                              v_sb[:, c, 0:otw])
                else:
                    for c in range(NCH):
                        otp = ps_pool.tile([128, otw], FP32, name="otp",
                                           tag="otp", bufs=2)
                        for g in range(4):
                            gh = g % 2
                            abd4 = abd_by_g[g][:].rearrange(
                                "p (j x) -> p j x", j=4)
                            for p in range(2):
                                s0 = gh * GW + (2 * c + p) * 32
                                nc.tensor.matmul(
                                    otp[64 * p:64 * (p + 1),
                                        g * 4 * qm:(g + 1) * 4 * qm],
                                    lhsT=v_sb[:, g,
                                              (2 * c + p) * 64:
                                              (2 * c + p + 1) * 64],
                                    rhs=abd4[:, :, s0:s0 + qm],
                                    tile_position=(0, 64 * p),
                                )
                        _split_evac(nc, ot_sb[:, c, :], otp, otw)
                ot_sbs.append(ot_sb)

            # out-projection, PAIRED on the wo stationary across the two
            # tiles; then accumulate into the SBUF y tiles
            for mc in range(NCH):
                ytag = "yps" if mc % 2 == 0 else "otp"
                ypss = [ps_pool.tile([128, otw], FP32, name="yps", tag=ytag,
                                     bufs=2) for _ in tails]
                for kc in range(NCH):
                    for ot_sb, yps in zip(ot_sbs, ypss):
                        nc.tensor.matmul(
                            yps[:],
                            lhsT=wo_sb[:, kc, 128 * mc:128 * (mc + 1)],
                            rhs=ot_sb[:, kc, :],
                            start=(kc == 0), stop=(kc == NCH - 1),
                        )
                for (it, _v, _a), yps in zip(tails, ypss):
                    y4d = y_sb[mc][:].rearrange("p (t h w) -> p t h w",
                                                t=T, h=HL)
                    if axis == "w":
                        # first pass: plain write, fold the (summed) bias;
                        # free-split across ACT halves the bank latency
                        dst = y_sb[mc][:, it * TILE:(it + 1) * TILE]
                        nc.scalar.activation(
                            dst[:, 0:256], yps[:, 0:256],
                            mybir.ActivationFunctionType.Identity,
                            bias=bias_aps[mc],
                        )
                        nc.scalar.activation(
                            dst[:, 256:512], yps[:, 256:512],
                            mybir.ActivationFunctionType.Identity,
                            bias=bias_aps[mc],
                        )
                    elif axis == "t":
                        # tile it covers h-row it; psum tokens are
                        # (w 32, t 16) t-fastest; y natural (t-major)
                        y_slice = y4d[:, :, it, :]            # (128, t16, w32)
                        yp3 = (yps[:].rearrange("p (w t) -> p w t", w=W)
                               .transpose([0, 2, 1]))
                        nc.vector.tensor_tensor(
                            y_slice, y_slice, yp3, mybir.AluOpType.add
                        )
                    else:
                        # h-pass: tile it covers t=it//2, w-half=it%2,
                        # tokens (tw 16, h 32) h-fastest; owned h is 0..15
                        t_idx, w_half = it // 2, it % 2
                        ws = slice(16 * w_half, 16 * (w_half + 1))
                        y_slice = y4d[:, t_idx, :, ws]        # (128, hl16, w16)
                        yp3 = (yps[:].rearrange("p (w h) -> p w h", w=16)
                               .transpose([0, 2, 1]))
                        nc.vector.tensor_tensor(
                            y_slice, y_slice, yp3, mybir.AluOpType.add
                        )

        pending.append(tail_pair)


def build_program():
    """Build + compile the SPMD bass program (same program on all 8 cores)."""
    nc = bacc.Bacc(
        "TRN2", target_bir_lowering=False, debug=False,
        enable_asserts=False, num_devices=N_CORES,
    )

    def din(name, shape, dt=BF16):
        return nc.dram_tensor(name, shape, dt, kind="ExternalInput").ap()

    x_w = din("x_w", (C, TOK_LOCAL))
    x_t = din("x_t", (C, TOK_LOCAL))
    x_h = din("x_h", (C, TOK_FULL))
    w_in = {}
    for ax in ("w", "t", "h"):
        for nm in ("wq", "wk", "wv", "wo"):
            w_in[f"{nm}_{ax}"] = din(f"{nm}_{ax}", (C, C))
    bias_in = din("bias", (C, 1), FP32)
    tml_in = din("tml", (2, 128))
    tmr_in = din("tmr", (2, 512))
    y_ap = nc.dram_tensor("y", (C, TOK_LOCAL), BF16, kind="ExternalOutput").ap()

    with tile.TileContext(nc) as tc:
        with contextlib.ExitStack() as ctx:
            xt_pool = ctx.enter_context(tc.tile_pool(name="xt", bufs=4))
            w_pool = ctx.enter_context(tc.tile_pool(name="wts", bufs=2))
            qk_pool = ctx.enter_context(tc.tile_pool(name="qk", bufs=2))
            v_pool = ctx.enter_context(tc.tile_pool(name="v", bufs=4))
            a_pool = ctx.enter_context(tc.tile_pool(name="a", bufs=3))
            sm_pool = ctx.enter_context(tc.tile_pool(name="sm", bufs=3))
            ot_pool = ctx.enter_context(tc.tile_pool(name="ot", bufs=2))
            ps_pool = ctx.enter_context(tc.tile_pool(name="ps", bufs=2, space="PSUM"))
            sps_pool = ctx.enter_context(tc.tile_pool(name="sps", bufs=2, space="PSUM"))
            const_pool = ctx.enter_context(tc.tile_pool(name="const", bufs=1))

            # constants
            tml_sb = const_pool.tile([2, 128], BF16)
            nc.sync.dma_start(tml_sb[:], tml_in[:])
            tmr_sb = const_pool.tile([2, 512], BF16)
            nc.sync.dma_start(tmr_sb[:], tmr_in[:])
            z_sb = const_pool.tile([1, 512], BF16)
            nc.gpsimd.memset(z_sb[:], 0.0)
            tc._z_sb = z_sb
            bias_sb = const_pool.tile([128, NCH], FP32)
            for mc in range(NCH):
                nc.sync.dma_start(
                    bias_sb[:, mc:mc + 1], bias_in[128 * mc:128 * (mc + 1), :]
                )
            bias_aps = [bias_sb[:, mc:mc + 1] for mc in range(NCH)]

            # persistent SBUF y accumulator: NCH chunk tiles (128, 8192) bf16
            y_sb = [const_pool.tile([128, TOK_LOCAL], BF16, name=f"y{mc}")
                    for mc in range(NCH)]

            # persistent block-diagonal A^T buffers (8-deep rotation: four
            # per supertile, tails deferred one supertile) and parity-split
            # k buffers, zeroed once
            abd_tiles = []
            for i in range(8):
                t = const_pool.tile([128, 4 * 512], BF16, name=f"abd{i}")
                nc.gpsimd.memset(t[:], 0.0)
                abd_tiles.append(t)
            tc._abd_flip = 0
            kz_tiles = []
            for i in range(2):
                t = const_pool.tile([128, 2, NCH, TILE], BF16, name=f"kz{i}")
                nc.gpsimd.memset(t[:], 0.0)
                kz_tiles.append(t)

            pools = (xt_pool, qk_pool, v_pool, a_pool, sm_pool,
                     ot_pool, ps_pool, sps_pool)

            pending = []
            for ax, x_ap in (("w", x_w), ("t", x_t), ("h", x_h)):
                w_aps = []
                for nm in ("wq", "wk", "wv", "wo"):
                    wt = w_pool.tile([128, NCH, C], BF16, tag=nm, name=nm)
                    for kc in range(NCH):
                        nc.sync.dma_start(
                            wt[:, kc, :],
                            w_in[f"{nm}_{ax}"][128 * kc:128 * (kc + 1), :],
                        )
                    w_aps.append(wt)
                _build_pass(tc, pools, ax, x_ap, w_aps, y_sb, bias_aps, tml_sb,
                            tmr_sb, kz_tiles, abd_tiles, pending)
            for t in pending:   # flush the last supertile's tails
                t()

            # final writeout: one DMA per feature chunk
            for mc in range(NCH):
                nc.sync.dma_start(
                    y_ap[128 * mc:128 * (mc + 1), :], y_sb[mc][:]
                )

    nc.compile()
    return nc


_PROGRAM = None


def _get_program():
    global _PROGRAM
    if _PROGRAM is None:
        _PROGRAM = build_program()
    return _PROGRAM


def make_in_maps(inputs):
    """Host-side shard + layout prep: per-core input dicts."""
    x = np.asarray(inputs["x"], np.float32)          # (B, C, T, H, W)
    scale = 1.0 / np.sqrt(D)

    weights = {}
    for ax in ("w", "h", "t"):
        for nm in ("wq", "wk", "wv", "wo"):
            wm = np.asarray(inputs[f"{nm}_{ax}"], np.float32)
            if nm == "wq":
                wm = wm * scale
            # lhsT layout: (C_in, C_out) = W.T
            weights[f"{nm}_{ax}"] = np.ascontiguousarray(wm.T).astype(BF16_NP)
    bias = (np.asarray(inputs["bo_w"], np.float32)
            + np.asarray(inputs["bo_h"], np.float32)
            + np.asarray(inputs["bo_t"], np.float32)).reshape(C, 1)

    # rank-2 additive cross-fiber mask for the t-pass:
    # S += tml.T @ tmr with tml one-hot on the query fiber and tmr = -60 on
    # cross-fiber key columns
    p = np.arange(128) % 32
    tml = np.stack([(p // 16) == e for e in range(2)]).astype(BF16_NP)
    f = np.arange(512) % 32
    tmr = np.stack([np.where((f // 16) != e, -60.0, 0.0) for e in range(2)]
                   ).astype(BF16_NP)

    in_maps = []
    for core in range(N_CORES):
        b, j = divmod(core, 2)
        xb = x[b]                                    # (C, T, H, W)
        xw = xb[:, :, 16 * j:16 * (j + 1), :]        # (C, T, HL, W) w-fastest
        xt = np.transpose(xw, (0, 2, 3, 1))          # (C, HL, W, T) t-fastest
        xh = np.transpose(xb, (0, 1, 3, 2))          # (C, T, W, H) h-fastest
        if j == 1:
            # rotate H so the owned half is always h-positions 0..15
            xh = np.concatenate([xh[..., 16:], xh[..., :16]], axis=-1)
        m = {
            "x_w": np.ascontiguousarray(xw).reshape(C, TOK_LOCAL).astype(BF16_NP),
            "x_t": np.ascontiguousarray(xt).reshape(C, TOK_LOCAL).astype(BF16_NP),
            "x_h": np.ascontiguousarray(xh).reshape(C, TOK_FULL).astype(BF16_NP),
            "bias": bias, "tml": tml, "tmr": tmr,
        }
        m.update(weights)
        in_maps.append(m)
    return in_maps


def assemble_output(results):
    """Gather per-core y (C, 8192) bf16 into (B, C, T, H, W) fp32."""
    out = np.empty((B, C, T, H, W), np.float32)
    for core in range(N_CORES):
        b, j = divmod(core, 2)
        y = np.asarray(results[core]["y"]).astype(np.float32).reshape(C, T, HL, W)
        out[b, :, :, 16 * j:16 * (j + 1), :] = y
    return out


_RUNNER = None


def _get_runner():
    """Build the sharded PJRT callable once; reuse across kernel() calls."""
    global _RUNNER
    if _RUNNER is not None:
        return _RUNNER
    import jax
    from jax.sharding import Mesh, PartitionSpec
    from jax.experimental.shard_map import shard_map
    from concourse import bass2jax

    nc = _get_program()
    bass2jax.install_neuronx_cc_hook()
    partition_name = (nc.partition_id_tensor.name
                      if nc.partition_id_tensor else None)
    in_names, out_names, out_avals, zero_outs = [], [], [], []
    for alloc in nc.m.functions[0].allocations:
        if not isinstance(alloc, mybir.MemoryLocationSet):
            continue
        name = alloc.memorylocations[0].name
        if alloc.kind == "ExternalInput":
            if name != partition_name:
                in_names.append(name)
        elif alloc.kind == "ExternalOutput":
            out_names.append(name)
            shape = tuple(alloc.tensor_shape)
            dtype = mybir.dt.np(alloc.dtype)
            out_avals.append(jax.core.ShapedArray(shape, dtype))
            zero_outs.append(np.zeros((N_CORES * shape[0], *shape[1:]), dtype))
    n_params = len(in_names)
    all_in_names = list(in_names) + out_names
    if partition_name is not None:
        all_in_names.append(partition_name)

    def _body(*args):
        operands = list(args)
        if partition_name is not None:
            operands.append(bass2jax.partition_id_tensor())
        return tuple(bass2jax._bass_exec_p.bind(
            *operands,
            out_avals=tuple(out_avals),
            in_names=tuple(all_in_names),
            out_names=tuple(out_names),
            lowering_input_output_aliases=(),
            sim_require_finite=True,
            sim_require_nnan=True,
            nc=nc,
        ))

    devices = jax.devices()[:N_CORES]
    mesh = Mesh(np.asarray(devices), ("core",))
    in_specs = (PartitionSpec("core"),) * (n_params + len(out_names))
    out_specs = (PartitionSpec("core"),) * len(out_names)
    fn = jax.jit(shard_map(_body, mesh=mesh, in_specs=in_specs,
                           out_specs=out_specs, check_rep=False))

    def run(in_maps):
        concat_in = [
            np.concatenate([np.asarray(in_maps[c][nm]) for c in range(N_CORES)],
                           axis=0)
            for nm in in_names
        ]
        outs = fn(*concat_in, *zero_outs)
        return [
            {nm: np.asarray(outs[i]).reshape(N_CORES, *out_avals[i].shape)[c]
             for i, nm in enumerate(out_names)}
            for c in range(N_CORES)
        ]

    _RUNNER = run
    return run


def kernel(**inputs) -> np.ndarray:
    run = _get_runner()
    in_maps = make_in_maps(inputs)
    return assemble_output(run(in_maps))
